# revision 11
# baseline (speedup 1.0000x reference)
"""AttnRNNCell Trainium2 kernel: 8-core data-parallel over batch.

Layout: feature-major ("transposed") activations [H, B_local] so chained
GEMMs need no per-matmul transposes. Host pre-transposes weights/inputs.
Wp/Wq/Wo GEMMs run in bf16; the gate GEMM runs in fp8e4m3 with
perf_mode=DoubleRow (2 contraction rows per PE cell = ~2x matmul rate),
using the algebraic form sigmoid(Wga h + Wgb ao + bg) (ao = gelu out has
~3x smaller magnitude than y = ao + h, which halves fp8 quantization
error; fp8 operands are pre-scaled x64 (weights) / x16 (acts), and the
activation read-out folds the 2^-10 compensation into its scale).

The gate's h-half (Wga.h) has no data dependence on the attention path, so
its matmuls are emitted right after the softmax combine loop: the Tile
scheduler uses them to keep the PE busy during the DVE-heavy softmax/
combine window. Its PSUM tiles are staged to SBUF (x1024-scaled bf16) and
re-injected into the gate PSUM accumulation by an identity matmul before
the Wgb.ao DoubleRow matmuls accumulate on top.

Elementwise work is split across DVE and Pool (GPSIMD) engines in the two
windows where DVE is the critical resource (attn combine, gate blend).
"""
import sys

sys.path.insert(0, "/opt/trn_rl_repo")

import numpy as np
import ml_dtypes

import concourse.bass as bass
import concourse.tile as tile
import concourse.mybir as mybir
from concourse.bass_utils import run_bass_kernel_spmd

F32 = mybir.dt.float32
F32R = mybir.dt.float32r
BF16 = mybir.dt.bfloat16
FP8 = mybir.dt.float8e4
AF = mybir.ActivationFunctionType
DR = mybir.MatmulPerfMode.DoubleRow
BF = ml_dtypes.bfloat16
F8 = ml_dtypes.float8_e4m3fn

B, IN, H, NH = 8192, 1024, 2048, 4
HD = H // NH
EPS = 1e-5
NCORES = 8
BL = B // NCORES          # 1024 batch rows per core
NB = BL // 512            # 2 N-slices of 512
NKH = H // 128            # 16 feature tiles for H-sized dims
NKI = IN // 128           # 8 feature tiles for IN
SCALE = 1.0 / float(np.sqrt(np.float32(HD)))
WS = 64.0                 # fp8 weight pre-scale
AS = 16.0                 # fp8 activation pre-scale
PSI = 1.0 / (WS * AS)     # PSUM read-out compensation (2^-10)

_DMA_OPS = ("InstDMACopy", "InstDMATranspose", "InstDMAMemset")


def _to_f32r(a):
    u = np.ascontiguousarray(a, dtype=np.float32).view(np.uint32)
    r = (u + 0x7FF + ((u >> 12) & 1)) & np.uint32(0xFFFFF000)
    return r.view(np.float32)


def _legalize_sync(nc, wait_cap=1, upd_cap=1):
    """This container's walrus supports ~1 sync wait/update per engine
    instruction; hoist the excess onto adjacent NoOps (same engine)."""
    ctr = [0]

    def mknop(eng, waits, upds):
        ctr[0] += 1
        nop = mybir.InstNoOp(name=f"lsync-{ctr[0]}", ins=[], outs=[])
        nop.engine = eng
        nop.sync_info = mybir.SyncInfo(on_wait=list(waits), on_update=list(upds))
        return nop

    for fn in nc.m.functions:
        for blk in fn.blocks:
            out = []
            changed = False
            for ins in blk.instructions:
                si = getattr(ins, "sync_info", None)
                if si is None:
                    out.append(ins)
                    continue
                waits = list(si.on_wait or [])
                upds = list(si.on_update or [])
                pre, post = [], []
                while len(waits) > wait_cap:
                    pre.append(mknop(ins.engine, [waits.pop(0)], []))
                if ins.__class__.__name__ not in _DMA_OPS:
                    while len(upds) > upd_cap:
                        post.append(mknop(ins.engine, [], [upds.pop()]))
                if pre or post:
                    ins.sync_info = mybir.SyncInfo(on_wait=waits, on_update=upds)
                    changed = True
                out.extend(pre)
                out.append(ins)
                out.extend(post)
            if changed:
                try:
                    blk.instructions = out
                except Exception:
                    blk.instructions.clear()
                    blk.instructions.extend(out)


def build():
    nc = bass.Bass()
    d = {}
    d["h"] = nc.dram_tensor("h", [H, BL], BF16, kind="ExternalInput")
    d["h8"] = nc.dram_tensor("h8", [H, BL], FP8, kind="ExternalInput")
    d["x"] = nc.dram_tensor("x", [IN, BL], BF16, kind="ExternalInput")
    d["wp"] = nc.dram_tensor("wp", [NKH, 128, NKI, 128], BF16, kind="ExternalInput")
    d["wq"] = nc.dram_tensor("wq", [NKH, 128, NKH, 128], BF16, kind="ExternalInput")
    d["wo"] = nc.dram_tensor("wo", [NKH, 128, NKH, 128], BF16, kind="ExternalInput")
    d["wga"] = nc.dram_tensor("wga", [NKH, 128, NKH, 128], FP8, kind="ExternalInput")
    d["wgb"] = nc.dram_tensor("wgb", [NKH, 128, NKH, 128], FP8, kind="ExternalInput")
    d["bvec"] = nc.dram_tensor("bvec", [128, 6, NKH], F32, kind="ExternalInput")
    d["oneseg"] = nc.dram_tensor("oneseg", [128, NH, NH], BF16, kind="ExternalInput")
    d["numk"] = nc.dram_tensor("numk", [NH, NH, 128], BF16, kind="ExternalInput")
    d["selg"] = nc.dram_tensor("selg", [NH, NH, 128], BF16, kind="ExternalInput")
    d["onesrow"] = nc.dram_tensor("onesrow", [1, 128], F32R, kind="ExternalInput")
    d["onescol"] = nc.dram_tensor("onescol", [128, 1], BF16, kind="ExternalInput")
    d["ident"] = nc.dram_tensor("ident", [128, 128], BF16, kind="ExternalInput")
    d["outT"] = nc.dram_tensor("outT", [H, BL], F32, kind="ExternalOutput")

    with tile.TileContext(nc) as tc:
        _body(nc, tc, d)
    _legalize_sync(nc)
    return nc


def _body(nc, tc, d):
    consts = tc.alloc_tile_pool(name="consts", bufs=1, side="left")
    p_h = tc.alloc_tile_pool(name="p_h", bufs=1, side="left")
    p_wr = tc.alloc_tile_pool(name="p_wr", bufs=2, side="left")
    p_tb = tc.alloc_tile_pool(name="p_tb", bufs=4, side="left")
    p_out = tc.alloc_tile_pool(name="p_out", bufs=2, side="left")

    def stream_w(pool, dram, nk, m, dt, tag):
        w = pool.tile([128, nk, 128], dt, tag=tag)
        nc.sync.dma_start(out=w[:, :, :], in_=dram[m, :, :, :])
        return w

    # biases first (single packed DMA, A's ACT needs them early), then x
    bvt = consts.tile([128, 6, NKH], F32)
    nc.sync.dma_start(out=bvt, in_=d["bvec"][:, :, :])
    bt = {n: bvt[:, i, :] for i, n in
          enumerate(("bp", "bq", "bo", "bg", "gam", "bet"))}
    p_x = tc.alloc_tile_pool(name="p_x", bufs=1, side="left")
    xT = p_x.tile([128, NKI, BL], BF16)
    for k in range(NKI):
        nc.sync.dma_start(out=xT[:, k, :], in_=d["x"][k * 128:(k + 1) * 128, :])

    ps_sB = tc.alloc_tile_pool(name="ps_sB", bufs=1, space="PSUM", side="left")
    ps_ab = tc.alloc_tile_pool(name="ps_ab", bufs=2, space="PSUM", side="left")

    # ---- A: xp = Wp x + bp  (bf16) ----
    p_xp = tc.alloc_tile_pool(name="p_xp", bufs=1, side="right")
    p_h8 = tc.alloc_tile_pool(name="p_h8", bufs=1, side="right")
    xp = p_xp.tile([128, NKH, BL], BF16)
    for m in range(NKH):
        w = stream_w(p_wr, d["wp"], NKI, m, BF16, "wr")
        po = ps_ab.tile([128, BL], F32, tag="po")
        for bs in range(NB):
            sl = slice(bs * 512, bs * 512 + 512)
            for k in range(NKI):
                nc.tensor.matmul(po[:, sl], w[:, k, :], xT[:, k, sl],
                                 start=(k == 0), stop=(k == NKI - 1))
        nc.scalar.activation(xp[:, m, :], po[:, :], AF.Identity,
                             bias=bt["bp"][:, m:m + 1], scale=1.0)

    # remaining consts (first needed at B's score matmuls / C / H2)
    oneseg = consts.tile([128, NH, NH], BF16)
    nc.sync.dma_start(out=oneseg, in_=d["oneseg"][:, :, :])
    numk = consts.tile([NH, NH, 128], BF16)
    nc.sync.dma_start(out=numk, in_=d["numk"][:, :, :])
    selg = consts.tile([NH, NH, 128], BF16)
    nc.sync.dma_start(out=selg, in_=d["selg"][:, :, :])
    onesrow = consts.tile([1, 128], F32R)
    nc.sync.dma_start(out=onesrow, in_=d["onesrow"][:, :])
    onescol = consts.tile([128, 1], BF16)
    nc.sync.dma_start(out=onescol, in_=d["onescol"][:, :])
    ident = consts.tile([128, 128], BF16)
    nc.sync.dma_start(out=ident, in_=d["ident"][:, :])
    epst = consts.tile([1, 1], F32)
    nc.vector.memset(epst, EPS)

    # h loads (needed from B on); emitted after A so A's DMAs go first
    hT = p_h.tile([128, NKH, BL], BF16)
    for k in range(NKH):
        nc.sync.dma_start(out=hT[:, k, :], in_=d["h"][k * 128:(k + 1) * 128, :])
    hT8 = p_h8.tile([128, NKH, BL], FP8)
    for k in range(NKH):
        nc.sync.dma_start(out=hT8[:, k, :], in_=d["h8"][k * 128:(k + 1) * 128, :])
    p_x.release()

    # ---- B: q GEMM + products + score reductions ----
    S = ps_sB.tile([128, BL], F32, tag="sps")   # rows 0-3: s0, 32-35: s1, 64-67: s3
    for m in range(NKH):
        g = m // NH
        w = stream_w(p_wr, d["wq"], NKH, m, BF16, "wr")
        po = ps_ab.tile([128, BL], F32, tag="po")
        for bs in range(NB):
            sl = slice(bs * 512, bs * 512 + 512)
            for k in range(NKH):
                nc.tensor.matmul(po[:, sl], w[:, k, :], hT[:, k, sl],
                                 start=(k == 0), stop=(k == NKH - 1))
        qm = p_tb.tile([128, BL], BF16, tag="tb", name="qm")
        nc.scalar.activation(qm[:, :], po[:, :], AF.Identity,
                             bias=bt["bq"][:, m:m + 1], scale=1.0)
        p0 = p_tb.tile([128, BL], BF16, tag="tb", name="p0")
        p1 = p_tb.tile([128, BL], BF16, tag="tb", name="p1")
        p3 = p_tb.tile([128, BL], BF16, tag="tb", name="p3")
        nc.vector.tensor_mul(p0[:, :], qm[:, :], hT[:, m, :])
        nc.vector.tensor_mul(p1[:, :], qm[:, :], xp[:, m, :])
        nc.vector.tensor_mul(p3[:, :], p0[:, :], xp[:, m, :])
        st, sp = (m == 0), (m == NKH - 1)
        for bs in range(NB):
            sl = slice(bs * 512, bs * 512 + 512)
            nc.tensor.matmul(S[0:4, sl], oneseg[:, g, :], p0[:, sl], start=st, stop=sp)
            nc.tensor.matmul(S[32:36, sl], oneseg[:, g, :], p1[:, sl], start=st, stop=sp)
            nc.tensor.matmul(S[64:68, sl], oneseg[:, g, :], p3[:, sl], start=st, stop=sp)
    ps_ab.release()

    # ---- C: softmax coefficients ----
    p_smA = tc.alloc_tile_pool(name="p_smA", bufs=1, side="right")
    E0 = p_smA.tile([4, BL], BF16)
    E1 = p_smA.tile([4, BL], BF16)
    E2 = p_smA.tile([4, BL], BF16)
    E3 = p_smA.tile([4, BL], BF16)
    nc.scalar.activation(E0[:, :], S[0:4, :], AF.Exp, scale=SCALE)
    nc.scalar.activation(E1[:, :], S[32:36, :], AF.Exp, scale=SCALE)
    nc.scalar.activation(E3[:, :], S[64:68, :], AF.Exp, scale=SCALE)
    nc.vector.tensor_mul(E2[:, :], E0[:, :], E1[:, :])   # exp(s0+s1) == e0*e1
    NUM = ps_sB.tile([128, BL], F32, tag="num")
    for qi, Eq in enumerate((E0, E1, E2, E3)):
        for bs in range(NB):
            sl = slice(bs * 512, bs * 512 + 512)
            nc.tensor.matmul(NUM[:, sl], numk[:, qi, :], Eq[:, sl],
                             start=(qi == 0), stop=(qi == 3))
    R = p_smA.tile([4, BL], F32)
    nc.vector.reciprocal(R[:, :], NUM[96:100, :])
    Ah = p_smA.tile([4, BL], BF16)
    Ax = p_smA.tile([4, BL], BF16)
    Az = p_smA.tile([4, BL], BF16)
    nc.vector.tensor_mul(Ah[:, :], NUM[0:4, :], R[:, :])
    nc.vector.tensor_mul(Ax[:, :], NUM[32:36, :], R[:, :])
    nc.vector.tensor_mul(Az[:, :], NUM[64:68, :], R[:, :])
    ps_sB.release()

    # ---- D1: attn combine. Coefficient grids are PE-broadcast per head,
    # then ACT-copied to bf16 SBUF so the whole combine chain runs in DVE
    # 2x (16-bit) mode; chunks round-robin DVE/DVE/DVE/Pool. The g1h
    # matmuls emitted right after keep the PE busy through this window ----
    p_g1h = tc.alloc_tile_pool(name="p_g1h", bufs=1, side="left")
    p_attn = tc.alloc_tile_pool(name="p_attn", bufs=1, side="left")
    p_td = tc.alloc_tile_pool(name="p_td", bufs=2, side="left")
    ps_cD = tc.alloc_tile_pool(name="ps_cD", bufs=1, space="PSUM", side="left")
    ps_h1 = tc.alloc_tile_pool(name="ps_h1", bufs=1, space="PSUM", side="left")
    attn = p_attn.tile([128, NKH, BL], BF16)
    ci = 0
    for g in range(NH):
        CH = ps_cD.tile([128, BL], F32, tag="CH")
        CX = ps_cD.tile([128, BL], F32, tag="CX")
        CZ = ps_cD.tile([128, BL], F32, tag="CZ")
        for bs in range(NB):
            sl = slice(bs * 512, bs * 512 + 512)
            nc.tensor.matmul(CH[:, sl], selg[:, g, :], Ah[:, sl], start=True, stop=True)
            nc.tensor.matmul(CX[:, sl], selg[:, g, :], Ax[:, sl], start=True, stop=True)
            nc.tensor.matmul(CZ[:, sl], selg[:, g, :], Az[:, sl], start=True, stop=True)
        CHb = p_td.tile([128, BL], BF16, tag="chb")
        CXb = p_td.tile([128, BL], BF16, tag="cxb")
        CZb = p_td.tile([128, BL], BF16, tag="czb")
        nc.scalar.activation(CHb[:, :], CH[:, :], AF.Identity, scale=1.0)
        nc.scalar.activation(CXb[:, :], CX[:, :], AF.Identity, scale=1.0)
        nc.scalar.activation(CZb[:, :], CZ[:, :], AF.Identity, scale=1.0)
        for m in range(g * NH, (g + 1) * NH):
            eng = nc.vector if (ci % 4) < 3 else nc.gpsimd
            ci += 1
            am = attn[:, m, :]
            t0 = p_td.tile([128, BL], BF16, tag="t0", name="t0")
            eng.tensor_mul(am, xp[:, m, :], CXb[:, :])
            eng.tensor_mul(t0[:, :], xp[:, m, :], CZb[:, :])
            eng.tensor_add(t0[:, :], t0[:, :], CHb[:, :])
            eng.tensor_mul(t0[:, :], t0[:, :], hT[:, m, :])
            eng.tensor_add(am, am, t0[:, :])

    # ---- H1: g1h = Wga h (fp8 DoubleRow), staged to SBUF x1024-scaled ----
    g1h = p_g1h.tile([128, NKH, BL], BF16)
    for m in range(NKH):
        w = stream_w(p_wr, d["wga"], NKH, m, FP8, "wr")
        po = ps_h1.tile([128, BL], F32, tag="po1")
        for bs in range(NB):
            sl = slice(bs * 512, bs * 512 + 512)
            for t in range(NKH // 2):
                nc.tensor.matmul(po[:, sl], w[:, 2 * t:2 * t + 2, :],
                                 hT8[:, 2 * t:2 * t + 2, sl],
                                 start=(t == 0), stop=(t == NKH // 2 - 1),
                                 perf_mode=DR)
        nc.scalar.activation(g1h[:, m, :], po[:, :], AF.Identity, scale=1.0)
    ps_h1.release()
    ps_cD.release()
    p_td.release()
    p_smA.release()
    p_h8.release()
    p_xp.release()

    # ---- D2 (+ fused F): ao = gelu(Wo attn + bo); y = ao + h; LN sums ----
    ps_de = tc.alloc_tile_pool(name="ps_de", bufs=2, space="PSUM", side="left")
    ps_sF = tc.alloc_tile_pool(name="ps_sF", bufs=1, space="PSUM", side="left")
    p_wb = tc.alloc_tile_pool(name="p_wb", bufs=3, side="right")
    p_y = tc.alloc_tile_pool(name="p_y", bufs=1, side="right")
    p_ao8 = tc.alloc_tile_pool(name="p_ao8", bufs=1, side="right")
    y = p_y.tile([128, NKH, BL], BF16)
    ao8 = p_ao8.tile([128, NKH, BL], FP8)
    SUM = ps_sF.tile([128, BL], F32, tag="sum")
    for m in range(NKH):
        w = stream_w(p_wb, d["wo"], NKH, m, BF16, "wb")
        po = ps_de.tile([128, BL], F32, tag="po")
        for bs in range(NB):
            sl = slice(bs * 512, bs * 512 + 512)
            for k in range(NKH):
                nc.tensor.matmul(po[:, sl], w[:, k, :], attn[:, k, sl],
                                 start=(k == 0), stop=(k == NKH - 1))
        ao = p_tb.tile([128, BL], BF16, tag="tb", name="ao")
        nc.scalar.activation(ao[:, :], po[:, :], AF.Gelu,
                             bias=bt["bo"][:, m:m + 1], scale=1.0)
        nc.scalar.activation(ao8[:, m, :], ao[:, :], AF.Identity, scale=AS)
        nc.vector.tensor_add(y[:, m, :], ao[:, :], hT[:, m, :])
        ysq = p_tb.tile([128, BL], BF16, tag="tb", name="ysq")
        nc.vector.tensor_mul(ysq[:, :], y[:, m, :], y[:, m, :])
        st, sp = (m == 0), (m == NKH - 1)
        for bs in range(NB):
            sl = slice(bs * 512, bs * 512 + 512)
            nc.tensor.matmul(SUM[0:1, sl], onescol[:, :], y[:, m, sl], start=st, stop=sp)
            nc.tensor.matmul(SUM[32:33, sl], onescol[:, :], ysq[:, sl], start=st, stop=sp)
    p_attn.release()

    # ---- G: mu / rstd rows + PE broadcast ----
    p_smB = tc.alloc_tile_pool(name="p_smB", bufs=1, side="right")
    MUr = p_smB.tile([1, BL], F32R)
    MSQ = p_smB.tile([1, BL], F32)
    nc.vector.tensor_scalar(out=MUr[:, :], in0=SUM[0:1, :], scalar1=1.0 / H,
                            scalar2=None, op0=mybir.AluOpType.mult)
    nc.vector.tensor_scalar(out=MSQ[:, :], in0=SUM[32:33, :], scalar1=1.0 / H,
                            scalar2=None, op0=mybir.AluOpType.mult)
    MUf = MUr.bitcast(F32)
    MU2 = p_smB.tile([1, BL], F32)
    nc.vector.tensor_mul(MU2[:, :], MUf[:, :], MUf[:, :])
    nc.vector.tensor_sub(MSQ[:, :], MSQ[:, :], MU2[:, :])
    nc.scalar.activation(MU2[:, :], MSQ[:, :], AF.Sqrt, bias=epst[:, 0:1], scale=1.0)
    nc.vector.reciprocal(MSQ[:, :], MU2[:, :])
    RSTr = p_smB.tile([1, BL], F32R)
    nc.vector.tensor_copy(RSTr[:, :], MSQ[:, :])
    ps_sF.release()

    ps_gh = tc.alloc_tile_pool(name="ps_gh", bufs=1, space="PSUM", side="left")
    MUB = ps_gh.tile([128, BL], F32, tag="mub")
    RSB = ps_gh.tile([128, BL], F32, tag="rsb")
    for bs in range(NB):
        sl = slice(bs * 512, bs * 512 + 512)
        nc.tensor.matmul(MUB[:, sl], onesrow[:, :], MUr[:, sl], start=True, stop=True)
        nc.tensor.matmul(RSB[:, sl], onesrow[:, :], RSTr[:, sl], start=True, stop=True)

    # ---- H2: gate = sigmoid((g1h + Wgb ao) * 2^-10 + bg); blend + out ----
    p_tf = tc.alloc_tile_pool(name="p_tf", bufs=4, side="left")
    for m in range(NKH):
        w = stream_w(p_wb, d["wgb"], NKH, m, FP8, "wb")
        po = ps_de.tile([128, BL], F32, tag="po")
        for bs in range(NB):
            sl = slice(bs * 512, bs * 512 + 512)
            # preload gate PSUM with the staged h-half via identity matmul
            nc.tensor.matmul(po[:, sl], ident[:, :], g1h[:, m, sl],
                             start=True, stop=False)
            for t in range(NKH // 2):
                nc.tensor.matmul(po[:, sl], w[:, 2 * t:2 * t + 2, :],
                                 ao8[:, 2 * t:2 * t + 2, sl],
                                 start=False, stop=(t == NKH // 2 - 1),
                                 perf_mode=DR)
        gm = p_tb.tile([128, BL], BF16, tag="tb", name="gm")
        nc.scalar.activation(gm[:, :], po[:, :], AF.Sigmoid,
                             bias=bt["bg"][:, m:m + 1], scale=PSI)
        t0 = p_tf.tile([128, BL], F32, tag="tf", name="n0")
        t1 = p_tf.tile([128, BL], F32, tag="tf", name="n1")
        nc.vector.tensor_sub(t0[:, :], y[:, m, :], MUB[:, :])
        nc.vector.tensor_mul(t0[:, :], t0[:, :], RSB[:, :])
        nc.scalar.activation(t1[:, :], t0[:, :], AF.Identity,
                             bias=bt["bet"][:, m:m + 1],
                             scale=bt["gam"][:, m:m + 1])
        nc.gpsimd.tensor_sub(t1[:, :], t1[:, :], hT[:, m, :])
        nc.gpsimd.tensor_mul(t1[:, :], t1[:, :], gm[:, :])
        ot = p_out.tile([128, BL], F32, tag="ot")
        nc.vector.tensor_add(ot[:, :], t1[:, :], hT[:, m, :])
        nc.sync.dma_start(out=d["outT"][m * 128:(m + 1) * 128, :], in_=ot[:, :])

    p_smB.release()
    p_ao8.release()
    p_y.release()
    p_wb.release()
    p_tf.release()
    p_g1h.release()
    p_out.release()
    p_tb.release()
    p_wr.release()
    p_h.release()
    consts.release()
    ps_gh.release()
    ps_de.release()


_NC = None


def _get_nc():
    global _NC
    if _NC is None:
        _NC = build()
    return _NC


def _consts_np():
    oneseg = np.zeros((128, NH, NH), np.float32)
    for g in range(NH):
        oneseg[:, g, g] = 1.0
    numk = np.zeros((NH, NH, 128), np.float32)   # [k=g, q, m]
    for g in range(NH):
        numk[g, 0, g] = 1.0          # e0 -> a_h num
        numk[g, 0, 96 + g] = 1.0     # e0 -> denom
        numk[g, 1, 32 + g] = 1.0     # e1 -> a_xp num
        numk[g, 1, 96 + g] = 1.0
        numk[g, 2, g] = 1.0          # e2 -> a_h num
        numk[g, 2, 32 + g] = 1.0     # e2 -> a_xp num
        numk[g, 2, 96 + g] = 1.0
        numk[g, 3, 64 + g] = 1.0     # e3 -> a_hxp num
        numk[g, 3, 96 + g] = 1.0
    selg = np.zeros((NH, NH, 128), np.float32)   # [k, g, m]
    for g in range(NH):
        selg[g, g, :] = 1.0
    return dict(
        oneseg=oneseg.astype(BF),
        numk=numk.astype(BF),
        selg=selg.astype(BF),
        onesrow=_to_f32r(np.ones((1, 128), np.float32)),
        onescol=np.ones((128, 1), np.float32).astype(BF),
        ident=np.eye(128, dtype=np.float32).astype(BF),
    )


def _vec16(v):
    return np.ascontiguousarray(np.asarray(v, np.float32).reshape(NKH, 128).T)


def _fp8(a, scale):
    return np.clip(np.asarray(a, np.float32) * scale, -240.0, 240.0).astype(F8)


def prepare_in_maps(h_prev, x, W_proj, b_proj, W_q, b_q, W_o, b_o, W_g, b_g,
                    gamma, beta):
    def _pack(wT):
        # [K, M] -> [m, p, k, c] contiguous (per-partition 8KB chunks)
        K_, M_ = wT.shape
        return np.ascontiguousarray(
            wT.reshape(K_ // 128, 128, M_ // 128, 128).transpose(2, 1, 0, 3))

    Wg = np.asarray(W_g, np.float32)
    shared = {
        "wp": _pack(np.asarray(W_proj, np.float32).T).astype(BF),
        "wq": _pack(np.asarray(W_q, np.float32).T).astype(BF),
        "wo": _pack(np.asarray(W_o, np.float32).T).astype(BF),
        "wga": _fp8(_pack(Wg[:, :H].T), WS),
        "wgb": _fp8(_pack(Wg[:, H:].T), WS),
        "bvec": np.ascontiguousarray(np.stack(
            [_vec16(v) for v in (b_proj, b_q, b_o, b_g, gamma, beta)], axis=1)),
    }
    shared.update(_consts_np())
    h2 = np.asarray(h_prev, np.float32).reshape(B, H)
    x2 = np.asarray(x, np.float32)
    in_maps = []
    for c in range(NCORES):
        m = dict(shared)
        hc = np.ascontiguousarray(h2[c * BL:(c + 1) * BL].T)
        m["h"] = hc.astype(BF)
        m["h8"] = _fp8(hc, AS)
        m["x"] = np.ascontiguousarray(x2[c * BL:(c + 1) * BL].T).astype(BF)
        in_maps.append(m)
    return in_maps


def run_device(in_maps, **kw):
    nc = _get_nc()
    return run_bass_kernel_spmd(nc, in_maps, core_ids=list(range(NCORES)), **kw)


_RUNNER = None


def _get_runner():
    """Custom sharded runner: per-core tensors sharded on the core axis,
    replicated weights/consts transferred once (not 8x)."""
    global _RUNNER
    if _RUNNER is not None:
        return _RUNNER
    import jax
    from jax.sharding import Mesh, PartitionSpec, NamedSharding
    try:
        from jax import shard_map as _sm
        shard_map = _sm.shard_map if hasattr(_sm, "shard_map") else _sm
    except Exception:
        from jax.experimental.shard_map import shard_map
    from concourse.bass2jax import _bass_exec_p, partition_id_tensor, \
        install_neuronx_cc_hook
    install_neuronx_cc_hook()

    nc = _get_nc()
    pid_name = nc.partition_id_tensor.name if nc.partition_id_tensor else None
    in_names, out_names, out_avals = [], [], []
    for alloc in nc.m.functions[0].allocations:
        if not isinstance(alloc, mybir.MemoryLocationSet):
            continue
        name = alloc.memorylocations[0].name
        if alloc.kind == "ExternalInput" and name != pid_name:
            in_names.append(name)
        elif alloc.kind == "ExternalOutput":
            out_names.append(name)
            out_avals.append(jax.core.ShapedArray(
                tuple(alloc.tensor_shape), mybir.dt.np(alloc.dtype)))
    bind_names = in_names + out_names + ([pid_name] if pid_name else [])
    sharded_names = {"h", "h8", "x"}

    def _body(*args):
        operands = list(args)
        operands.append(partition_id_tensor())
        return tuple(_bass_exec_p.bind(
            *operands,
            out_avals=tuple(out_avals),
            in_names=tuple(bind_names),
            out_names=tuple(out_names),
            lowering_input_output_aliases=(),
            sim_require_finite=True,
            sim_require_nnan=True,
            nc=nc,
        ))

    devices = jax.devices()[:NCORES]
    mesh = Mesh(np.asarray(devices), ("core",))
    Pc, Pr = PartitionSpec("core"), PartitionSpec()
    in_specs = tuple(Pc if n in sharded_names else Pr for n in in_names) \
        + (Pc,) * len(out_names)
    import inspect
    _smkw = {}
    try:
        _p = inspect.signature(shard_map).parameters
        _smkw["check_rep" if "check_rep" in _p else "check_vma"] = False
    except Exception:
        _smkw["check_rep"] = False
    fn = jax.jit(
        shard_map(_body, mesh=mesh, in_specs=in_specs,
                  out_specs=(Pc,) * len(out_names), **_smkw),
        keep_unused=True)
    dev_zeros = [
        jax.device_put(
            np.zeros((NCORES * av.shape[0], *av.shape[1:]), av.dtype),
            NamedSharding(mesh, Pc))
        for av in out_avals
    ]
    _RUNNER = (fn, mesh, in_names, out_names, out_avals, sharded_names, dev_zeros)
    return _RUNNER


def run_device_fast(in_maps):
    fn, mesh, in_names, out_names, out_avals, sharded_names, dev_zeros = _get_runner()
    args = []
    for n in in_names:
        if n in sharded_names:
            args.append(np.concatenate([np.asarray(m[n]) for m in in_maps], axis=0))
        else:
            args.append(np.asarray(in_maps[0][n]))
    args.extend(dev_zeros)
    outs = fn(*args)
    return {name: np.asarray(outs[i]) for i, name in enumerate(out_names)}


def kernel(**inputs):
    in_maps = prepare_in_maps(**inputs)
    try:
        outs = run_device_fast(in_maps)
        big = outs["outT"].reshape(NCORES, H, BL)
        out = np.empty((B, H), np.float32)
        for c in range(NCORES):
            out[c * BL:(c + 1) * BL] = big[c].T
    except Exception:
        res = run_device(in_maps)
        out = np.empty((B, H), np.float32)
        for c in range(NCORES):
            out[c * BL:(c + 1) * BL] = res.results[c]["outT"].T
    return out.reshape(B, 1, H)
